# revision 1
# baseline (speedup 1.0000x reference)
"""DeltaHebbianBlock Trainium2 kernel (v2).

Sharding: 8 cores = (B=2) x (H=4) head-parallel. Each core computes its
head's delta-rule chunked scan (C=128 chunks) and the partial output
projection partial_bh = (o_bh @ Wr_h^T) (8192 x 1024, f32). Host gathers:
out[b] = x[b] + sum_h alpha_h * partial[b,h].

Per-core pipeline (T=8192, d=256, C=128, 64 windows, 8 passes of 1024):
  P1: xT8 (fp8, host-pretransposed) + xh (bf16) DMA; rk = normalize(xh);
      rkT via PE transpose (zero col 0 so col t == wk_t); wk shift via
      SBUF DMA; wkgN = wk*gamma^(127-p); rkgTn = rkT * -gamma^p.
  P2 per window: wk-gram + intra-gram from rkT; depth-2 UT chain
      AT = I + C0 + C1 + C0@C1 (C0=A0^T, C1=(A0^2)^T) with +I folded
      into identity matmuls; A_sb = AT^T via PE; W2T = A^T @ wkgN.
  P3 per window (scan): u = v + wk@Sneg in one PSUM accumulation
      (fp8 DoubleRow v-proj + bf16 wk@Sneg); sup = W2T^T @ u;
      Sneg = gC*Sneg - sup (split STT); vnew = A@u; oT = Sneg@(-rkgT)
      + vnew^T@intraT accumulated transposed in PSUM -> fp8 oT8.
  P4 per window: partial = oT8 @ wrt8 (fp8 DoubleRow) -> f32 PSUM ->
      direct DMA to DRAM.
"""
import os
import numpy as np
import ml_dtypes
from contextlib import ExitStack

import concourse.bass as bass
import concourse.mybir as mybir
import concourse.tile as tile
from concourse import bacc, bass_utils

B, T, D = 2, 8192, 1024
H, d, C = 4, 256, 128
NQ = 8                # passes
QT = T // NQ          # 1024 tokens per pass
QW = QT // C          # 8 windows per pass

F32 = mybir.dt.float32
BF16 = mybir.dt.bfloat16
FP8 = mybir.dt.float8e4

AF = mybir.ActivationFunctionType
ALU = mybir.AluOpType
DR = mybir.MatmulPerfMode.DoubleRow

# engine assignment per op: v=DVE, g=Pool(gpsimd), s=Act(scalar)
ENG = {
    "u": "v", "stt0": "v", "vnew": "s", "oT8": "s",
    "st0": "v", "st1": "s", "sq": "s", "rk": "v", "rkt0": "v",
    "rkgTn": "g", "B0": "g", "C0": "g", "inT": "v", "C1": "s", "AT": "v",
    "gsb": "v",
    "A_sb": "v", "W2T": "v",
}
for kv in os.environ.get("K_ENG", "").split(","):
    if "=" in kv:
        k, v = kv.split("=")
        ENG[k] = v


def _build():
    nc = bacc.Bacc("TRN2", target_bir_lowering=False, debug=False,
                   num_devices=int(os.environ.get("K_NCORES", "8")))
    xt8_d = nc.dram_tensor("xt8", (D, T), FP8, kind="ExternalInput")
    xh_d = nc.dram_tensor("xh", (T, d), BF16, kind="ExternalInput")
    wwt8_d = nc.dram_tensor("wwt8", (D, d), FP8, kind="ExternalInput")
    wrt8_d = nc.dram_tensor("wrt8", (d, D), FP8, kind="ExternalInput")
    mb_d = nc.dram_tensor("mb", (128, 128), BF16, kind="ExternalInput")
    mc_d = nc.dram_tensor("mc", (128, 128), BF16, kind="ExternalInput")
    mit_d = nc.dram_tensor("mit", (128, 128), BF16, kind="ExternalInput")
    id_d = nc.dram_tensor("ident", (128, 128), BF16, kind="ExternalInput")
    gpbn_d = nc.dram_tensor("gpbn", (128, QT), BF16, kind="ExternalInput")
    gptn_d = nc.dram_tensor("gptn", (128, 1), F32, kind="ExternalInput")
    gptin_d = nc.dram_tensor("gptin", (128, 128), BF16, kind="ExternalInput")
    gcid_d = nc.dram_tensor("gcid", (128, 128), BF16, kind="ExternalInput")
    part_d = nc.dram_tensor("partial", (T, D), BF16, kind="ExternalOutput")

    with ExitStack() as ctx:
        tc = ctx.enter_context(tile.TileContext(nc))
        consts = ctx.enter_context(tc.tile_pool(name="consts", bufs=1))
        big = ctx.enter_context(tc.tile_pool(name="big", bufs=1))
        pp = ctx.enter_context(tc.tile_pool(name="pp", bufs=2))
        chp = ctx.enter_context(tc.tile_pool(name="chp", bufs=2))
        sm = ctx.enter_context(tc.tile_pool(name="sm", bufs=3))
        scr = ctx.enter_context(tc.tile_pool(name="scr", bufs=2))
        ps_g = ctx.enter_context(tc.tile_pool(name="ps_g", bufs=int(os.environ.get("K_PSG", "3")), space="PSUM"))
        ps_t = ctx.enter_context(tc.tile_pool(name="ps_t", bufs=1, space="PSUM"))
        ps_a = ctx.enter_context(tc.tile_pool(name="ps_a", bufs=2, space="PSUM"))
        ps_h = ctx.enter_context(tc.tile_pool(name="ps_h", bufs=int(os.environ.get("K_PSH", "1")), space="PSUM"))
        ps_s = ctx.enter_context(tc.tile_pool(name="ps_s", bufs=1, space="PSUM"))

        # ---- constants / weights in SBUF ----
        wwt8_s = consts.tile([128, 8, d], FP8)
        nc.sync.dma_start(wwt8_s[:], wwt8_d.ap().rearrange("(kb p) j -> p kb j", p=128))
        wrt8_s = consts.tile([128, 2, D], FP8)
        nc.sync.dma_start(wrt8_s[:], wrt8_d.ap().rearrange("(kt p) n -> p kt n", p=128))
        mb_s = consts.tile([128, 128], BF16)
        nc.sync.dma_start(mb_s[:], mb_d.ap())
        mc_s = consts.tile([128, 128], BF16)
        nc.sync.dma_start(mc_s[:], mc_d.ap())
        mit_s = consts.tile([128, 128], BF16)
        nc.sync.dma_start(mit_s[:], mit_d.ap())
        id_s = consts.tile([128, 128], BF16)
        nc.sync.dma_start(id_s[:], id_d.ap())
        gpbn_s = consts.tile([128, QT], BF16)
        nc.sync.dma_start(gpbn_s[:], gpbn_d.ap())
        gptn_s = consts.tile([128, 1], F32)
        nc.sync.dma_start(gptn_s[:], gptn_d.ap())
        gptin_s = consts.tile([128, 128], BF16)
        nc.sync.dma_start(gptin_s[:], gptin_d.ap())
        gcid_s = consts.tile([128, 128], BF16)
        nc.sync.dma_start(gcid_s[:], gcid_d.ap())

        # ---- persistent ----
        rk = big.tile([128, T // 128, d], BF16)       # 4MB token-major
        rkT = big.tile([128, 2, T + 1], BF16)         # col 0 = zero pad
        Sneg = big.tile([128, 2, d], BF16)            # -S state
        nc.gpsimd.memset(Sneg[:], 0.0)
        nc.gpsimd.memset(rkT[:, :, 0:1], 0.0)

        # Software-pipelined emission at window granularity. Slot s emits,
        # in per-engine dependency-resolution order: scan chain of window
        # s-3, P4+output of s-4, v-proj of s-2, P2-tail of s-1, P1+P2-head
        # of s. Engines execute their FIFO in program order, so emission
        # order per engine must match the order dependencies resolve.
        LAG = int(os.environ.get("K_LAG", "3"))
        NW = NQ * QW
        ppt = {}    # pass-level tiles, keyed by pass index
        wtile = {}  # per-window tiles
        ups_t = {}
        st_t = {}
        wk_t = {}
        chn_t = {}

        EMAP = {"v": nc.vector, "g": nc.gpsimd, "s": nc.scalar}

        def E(key):
            return EMAP[ENG[key]]

        def CP(key, out, in_):
            if ENG[key] == "s":
                nc.scalar.activation(out, in_, AF.Copy)
            else:
                E(key).tensor_copy(out, in_)

        def SCALE(key, out, in_, sc):
            if ENG[key] == "s":
                nc.scalar.activation(out, in_, AF.Copy, scale=sc)
            else:
                E(key).tensor_scalar_mul(out, in_, sc)

        def p1_pass_head(q):
            qt0 = q * QT
            xT8 = pp.tile([128, 8, QT], FP8, tag="xT8", name=f"xT8_{q}")
            for hq in range(2):
                nc.sync.dma_start(
                    xT8[:, :, hq * 512:(hq + 1) * 512],
                    xt8_d.ap()[:, qt0 + hq * 512: qt0 + (hq + 1) * 512]
                    .rearrange("(kb p) t -> p kb t", p=128))
            xh = pp.tile([128, QW, d], BF16, tag="xh", name=f"xh_{q}")
            nc.sync.dma_start(
                xh[:], xh_d.ap()[qt0:qt0 + QT, :].rearrange("(tt p) j -> p tt j", p=128))
            inT = pp.tile([128, QT], BF16, tag="inT", name=f"inT_{q}")
            AT = pp.tile([128, QT], BF16, tag="AT", name=f"AT_{q}")
            W2T = pp.tile([128, QW, 256], BF16, tag="W2T", name=f"W2T_{q}")
            rkgTn = pp.tile([128, 2, QT], BF16, tag="rkgTn", name=f"rkgTn_{q}")
            oT8 = pp.tile([128, 2, QT], FP8, tag="oT8", name=f"oT8_{q}")
            st = pp.tile([128, QW, D], BF16, tag="st", name=f"st_{q}")
            ppt[q] = dict(xT8=xT8, xh=xh, inT=inT, AT=AT, W2T=W2T, rkgTn=rkgTn,
                          oT8=oT8, st=st)

        def slot(s):
            w3 = s - LAG        # scan-chain window
            wp = s - LAG - 1    # P4/output window
            v = s - 2           # v-proj window
            if s % QW == 0 and s < NW:
                p1_pass_head(s // QW)
            qs, ws = divmod(s, QW)
            q3, wl3 = (divmod(w3, QW)) if w3 >= 0 else (None, None)
            qp, wlp = (divmod(wp, QW)) if wp >= 0 else (None, None)
            qv, wv = (divmod(v, QW)) if v >= 0 else (None, None)

            # --- chain front (w3): wkS, gammaS, term1 all ready at t0 ---
            if 0 <= w3 < NW:
                t3 = ppt[q3]
                w0 = w3 * 128
                wl = wl3 * 128
                ups = ups_t[w3]
                for kt in range(2):
                    nc.tensor.matmul(ups[:, 0:256], rkT[:, kt, w0:w0 + 128],
                                     Sneg[:, kt, :], start=False, stop=(kt == 1))
                sup = ps_s.tile([128, 512], F32, tag="s")
                for jb in range(2):
                    nc.tensor.matmul(sup[:, jb * 256:(jb + 1) * 256], gcid_s[:],
                                     Sneg[:, jb, :], start=True, stop=False)
                for half in range(2):
                    for jb in range(2):
                        nc.tensor.matmul(ups[:, 256 + half * 128:256 + (half + 1) * 128],
                                         Sneg[:, jb, half * 128:(half + 1) * 128],
                                         t3["rkgTn"][:, jb, wl:wl + 128],
                                         start=(jb == 0), stop=False)
                u_sb = sm.tile([128, d], BF16, tag="u")
                E("u").tensor_copy(u_sb[:], ups[:, 0:256])
                chn_t[w3] = (u_sb, sup)

            # --- P4 of window wp ---
            if 0 <= wp < NW:
                tp = ppt[qp]
                for nh in range(2):
                    pp4 = ps_a.tile([128, 512], F32, tag="a")
                    nc.tensor.matmul(pp4[:], tp["oT8"][:, 0:2, wlp * 128:(wlp + 1) * 128],
                                     wrt8_s[:, 0:2, nh * 512:(nh + 1) * 512],
                                     perf_mode=DR)
                    CP("st1" if nh else "st0",
                       tp["st"][:, wlp, nh * 512:(nh + 1) * 512], pp4[:])
                if wlp % 2 == 1:
                    nc.sync.dma_start(
                        part_d.ap()[qp * QT + (wlp - 1) * 128: qp * QT + (wlp + 1) * 128, :]
                        .rearrange("(wq p) n -> p wq n", p=128),
                        tp["st"][:, wlp - 1:wlp + 1, :])

            # --- v-proj of window v ---
            if 0 <= v < NW:
                ups = ps_a.tile([128, 512], F32, tag="a", name=f"ups_{v}")
                ups_t[v] = ups
                for k in range(4):
                    nc.tensor.matmul(
                        ups[:, 0:256],
                        ppt[qv]["xT8"][:, 2 * k:2 * k + 2, wv * 128:(wv + 1) * 128],
                        wwt8_s[:, 2 * k:2 * k + 2, :],
                        start=(k == 0), stop=False, perf_mode=DR)

            # --- normalize head of window s ---
            if s < NW:
                t = ppt[qs]
                ss = scr.tile([128, 1], F32, tag="ss")
                if ENG["sq"] == "v":
                    sq = scr.tile([128, d], BF16, tag="sq")
                    nc.vector.tensor_tensor_reduce(sq[:], t["xh"][:, ws, :],
                                                   t["xh"][:, ws, :], 1.0, 0.0,
                                                   ALU.mult, ALU.add,
                                                   accum_out=ss[:])
                elif ENG["sq"] == "g":
                    sq = scr.tile([128, d], F32, tag="sq")
                    nc.gpsimd.tensor_mul(sq[:], t["xh"][:, ws, :], t["xh"][:, ws, :])
                    nc.gpsimd.tensor_reduce(ss[:], sq[:], mybir.AxisListType.X,
                                            ALU.add)
                else:
                    sq = scr.tile([128, d], F32, tag="sq")
                    nc.scalar.activation(sq[:], t["xh"][:, ws, :], AF.Square,
                                         accum_out=ss[:])
                nrm = scr.tile([128, 1], F32, tag="nrm")
                nc.scalar.activation(nrm[:], ss[:], AF.Sqrt)
                inv = scr.tile([128, 1], F32, tag="inv")
                nc.vector.reciprocal(inv[:], nrm[:])
                SCALE("rk", rk[:, s, :], t["xh"][:, ws, :], inv[:])

            # --- P2b of window s-1: C0, c1p, C1, ips, inT ---
            if 0 <= s - 1 < NW:
                w2 = s - 1
                q2, wl2 = divmod(w2, QW)
                t2 = ppt[q2]
                w0b = w2 * 128
                gsb, B0 = wk_t.pop(("p2a", w2))
                C0 = chp.tile([128, 128], BF16, tag="C0")
                E("C0").tensor_mul(C0[:], gsb[:], mc_s[:])
                c1p = ps_g.tile([128, 128], F32, tag="g")
                nc.tensor.matmul(c1p[:], B0[:], C0[:])
                C1 = chp.tile([128, 128], BF16, tag="C1")
                CP("C1", C1[:], c1p[:])
                ips = ps_g.tile([128, 128], F32, tag="g")
                for kt in range(2):
                    nc.tensor.matmul(ips[:], rkT[:, kt, w0b:w0b + 128],
                                     rkT[:, kt, w0b + 1:w0b + 129],
                                     start=(kt == 0), stop=(kt == 1))
                E("inT").tensor_mul(t2["inT"][:, wl2 * 128:(wl2 + 1) * 128], ips[:],
                                    mit_s[:])
                wk_t[("p2b", w2)] = (B0, C0, C1)

            # --- P2c of window s-2: g1u, A_sb, g1p, AT, w2p, W2T ---
            if 0 <= s - 2 < NW:
                wc = s - 2
                qc, wlc = divmod(wc, QW)
                tc_ = ppt[qc]
                B0c, C0c, C1c = wk_t.pop(("p2b", wc))
                g1u = ps_h.tile([128, 512], F32, tag="h")
                nc.tensor.matmul(g1u[:, 0:128], C1c[:], B0c[:], start=True, stop=False)
                nc.tensor.matmul(g1u[:, 0:128], C1c[:], id_s[:], start=False, stop=False)
                nc.tensor.matmul(g1u[:, 0:128], id_s[:], B0c[:], start=False, stop=True)
                A_sb = chp.tile([128, 128], BF16, tag="A_sb")
                nc.vector.scalar_tensor_tensor(A_sb[:], g1u[:, 0:128], gptn_s[:],
                                               gptin_s[:], ALU.mult, ALU.add)
                g1p = ps_h.tile([128, 512], F32, tag="h")
                nc.tensor.matmul(g1p[:, 0:128], B0c[:], C1c[:], start=True, stop=False)
                nc.tensor.matmul(g1p[:, 0:128], id_s[:], C1c[:], start=False, stop=False)
                nc.tensor.matmul(g1p[:, 0:128], id_s[:], C0c[:], start=False, stop=True)
                E("AT").tensor_add(tc_["AT"][:, wlc * 128:(wlc + 1) * 128],
                                   g1p[:, 0:128], id_s[:])
                wkw = wk_t.pop(("wkw", wc))
                w2p = ps_h.tile([128, 512], F32, tag="h")
                for jb in range(2):
                    nc.tensor.matmul(w2p[:, jb * 128:(jb + 1) * 128], A_sb[:],
                                     wkw[:, jb * 128:(jb + 1) * 128])
                CP("W2T", tc_["W2T"][:, wlc, :], w2p[:, 0:256])

            # --- chain middle (w3): sup stop, vnew mm, Sneg copies, vnew ---
            if 0 <= w3 < NW:
                u_sb, sup = chn_t[w3]
                ups = ups_t[w3]
                wl = wl3 * 128
                for jb in range(2):
                    nc.tensor.matmul(sup[:, jb * 256:(jb + 1) * 256],
                                     t3["W2T"][:, wl3, jb * 128:(jb + 1) * 128],
                                     u_sb[:], start=False, stop=True)
                nc.tensor.matmul(ups[:, 0:256], t3["AT"][:, wl:wl + 128], u_sb[:])
                E("stt0").tensor_copy(Sneg[:, 0, :], sup[:, 0:256])
                nc.scalar.activation(Sneg[:, 1, :], sup[:, 256:512], AF.Copy)
                vnew = sm.tile([128, d], BF16, tag="vn")
                CP("vnew", vnew[:], ups[:, 0:256])
                chn_t[w3] = (u_sb, sup, vnew)

            # --- P1 tail of window s: transposes, rkT, rkgTn, wkw DMAs ---
            if s < NW:
                tps = ps_t.tile([128, 256], BF16, tag="t")
                for kt in range(2):
                    nc.tensor.transpose(tps[:, kt * 128:(kt + 1) * 128],
                                        rk[:, s, kt * 128:(kt + 1) * 128], id_s[:])
                CP("rkt0", rkT[:, 0:2, 1 + s * 128: 1 + (s + 1) * 128], tps[:])
                for kt in range(2):
                    E("rkgTn").tensor_mul(
                        ppt[qs]["rkgTn"][:, kt, ws * 128:(ws + 1) * 128],
                        rkT[:, kt, 1 + s * 128: 1 + (s + 1) * 128],
                        gpbn_s[:, ws * 128:(ws + 1) * 128])
                wkw = scr.tile([128, d], BF16, tag="wkw", bufs=4)
                nc.sync.dma_start(wkw[1:128, :], rk[0:127, s, :])
                if s == 0:
                    nc.gpsimd.memset(wkw[0:1, :], 0.0)
                else:
                    nc.sync.dma_start(wkw[0:1, :], rk[127:128, s - 1, :])
                wk_t[("wkw", s)] = wkw

            # --- P2a of window s: gram + B0 ---
            if s < NW:
                w0 = s * 128
                gps = ps_g.tile([128, 128], F32, tag="g")
                for kt in range(2):
                    nc.tensor.matmul(gps[:], rkT[:, kt, w0:w0 + 128],
                                     rkT[:, kt, w0:w0 + 128],
                                     start=(kt == 0), stop=(kt == 1))
                gsb = chp.tile([128, 128], F32, tag="gsb")
                CP("gsb", gsb[:], gps[:])
                B0 = chp.tile([128, 128], BF16, tag="B0")
                E("B0").tensor_mul(B0[:], gsb[:], mb_s[:])
                wk_t[("p2a", s)] = (gsb, B0)

            # --- chain tail (w3): term2, oT8 copy ---
            if 0 <= w3 < NW:
                u_sb, sup, vnew = chn_t.pop(w3)
                ups = ups_t.pop(w3)
                wl = wl3 * 128
                for half in range(2):
                    nc.tensor.matmul(ups[:, 256 + half * 128:256 + (half + 1) * 128],
                                     vnew[:, half * 128:(half + 1) * 128],
                                     t3["inT"][:, wl:wl + 128], start=False, stop=True)
                CP("oT8", t3["oT8"][:, 0:2, wl:wl + 128], ups[:, 256:512])

        for s in range(NW + LAG + 1):
            slot(s)
    nc.compile()
    return nc


_NC = None
LAST_EXEC_NS = None
LAST_TRACE = None


def _bf16(a):
    return np.ascontiguousarray(a.astype(ml_dtypes.bfloat16))


def _fp8(a):
    return np.ascontiguousarray(a.astype(ml_dtypes.float8_e4m3fn))


def make_in_maps(out, Ww, Wr, decay, log_alpha, ncore=8):
    gamma = 1.0 / (1.0 + np.exp(-decay.astype(np.float64)))
    p128 = np.arange(128)
    in_maps = []
    for c in range(ncore):
        b, h = c // 4, c % 4
        g = gamma[h]
        xr = np.roll(out[b], -h * d, axis=1)
        wwr = np.roll(Ww[h * d:(h + 1) * d, :], -h * d, axis=1).T  # (D, d)
        wrs = Wr[:, h * d:(h + 1) * d].T                           # (d, D)
        Ls = np.tril(g ** np.maximum(p128[:, None] - p128[None, :], 0), -1)
        mb = (-Ls).astype(np.float32)                 # B0 = gram*mb = A0
        mit = np.triu(g ** np.maximum(p128[None, :] - p128[:, None], 0), 1
                      ).astype(np.float32)
        gp = (g ** p128).astype(np.float32)
        gpbn = np.tile(-gp, QT // 128)[None, :].repeat(128, 0)
        gptv = (g ** (127 - p128)).astype(np.float32)
        in_maps.append({
            "xt8": _fp8(xr.T),
            "xh": _bf16(xr[:, 0:d]),
            "wwt8": _fp8(wwr),
            "wrt8": _fp8(wrs),
            "mb": _bf16(mb), "mc": _bf16(mb.T),
            "mit": _bf16(mit),
            "ident": _bf16(np.eye(128, dtype=np.float32)),
            "gpbn": _bf16(gpbn),
            "gptn": (-gptv)[:, None].astype(np.float32),
            "gptin": _bf16(np.diag(-gptv)),
            "gcid": _bf16(np.eye(128, dtype=np.float32) * (g ** 128)),
        })
    return in_maps


def kernel(out, Ww, Wr, decay, log_alpha):
    global _NC
    out = np.asarray(out, dtype=np.float32)
    Ww = np.asarray(Ww, dtype=np.float32)
    Wr = np.asarray(Wr, dtype=np.float32)
    decay = np.asarray(decay, dtype=np.float32)
    log_alpha = np.asarray(log_alpha, dtype=np.float32)
    alpha = np.exp(log_alpha.astype(np.float64)).astype(np.float32)

    if _NC is None:
        _NC = _build()
    nc = _NC

    ncore = int(os.environ.get("K_NCORES", "8"))
    in_maps = make_in_maps(out, Ww, Wr, decay, log_alpha, ncore)
    res = bass_utils.run_bass_kernel_spmd(
        nc, in_maps, core_ids=list(range(ncore)),
        trace=bool(os.environ.get("K_TRACE")))
    global LAST_EXEC_NS, LAST_TRACE
    LAST_EXEC_NS = res.exec_time_ns
    LAST_TRACE = res.instructions_and_trace
    final = out.copy()
    for c in range(len(res.results)):
        b, h = c // 4, c % 4
        final[b] += alpha[h] * res.results[c]["partial"].astype(np.float32)
    return final

